# revision 7
# baseline (speedup 1.0000x reference)
"""Trainium2 Bass kernel for EvaAttention (B=4, S=2048, C=1024, H=16, D=64).

Sharding: 8 cores = 4 batches x 2 head-groups (8 heads each). Each core runs
the identical SPMD program on host-sliced inputs:
  - qk-projection in [chan, seq] layout (2 heads per 128-partition tile),
    RoPE applied via a permutation matmul (rotate-half) + fused DVE ops,
  - v-projection in [seq, chan] layout with a packed ones-column so the
    attention matmul also produces the softmax denominators,
  - per-head  exp(QK^T) -> AV accumulate -> normalize,
  - output projection producing the per-core partial y^T.
Host sums the two head-group partials per batch and adds the bias
corrections (proj bias + v_bias folded through the projection).
"""

import os
import sys

import numpy as np

for _p in ("/opt/trn_rl_repo", "/root/.axon_site/_ro/trn_rl_repo"):
    if os.path.isdir(_p) and _p not in sys.path:
        sys.path.append(_p)

import concourse.bass as bass  # noqa: E402,F401
import concourse.mybir as mybir  # noqa: E402
import concourse.tile as tile  # noqa: E402
from concourse import bacc  # noqa: E402
from concourse.bass_utils import run_bass_kernel_spmd  # noqa: E402

F32 = mybir.dt.float32
F32R = mybir.dt.float32r
AF = mybir.ActivationFunctionType
OP = mybir.AluOpType

B = 4
C = 1024
D = 64
H = 16
HPC = 8  # heads per core
NCORES = 8
KC = C // 128  # contraction chunks for the projections
VW = D + 1  # v-store block width per head (64 v cols + ones col)


def _nchunks(width):
    """Split a free-dim width into <=512 column chunks."""
    out = []
    n0 = 0
    while n0 < width:
        nn = min(512, width - n0)
        out.append((n0, nn))
        n0 += nn
    return out


def _emit(tc, io, S):
    nc = tc.nc
    KT = S // 128  # k-position tiles
    S2 = S // 2  # psum half width for pipelining
    half_chunks = _nchunks(S2)
    full_chunks = _nchunks(S)

    r32 = lambda ap: ap.bitcast(F32R)  # noqa: E731

    with (
        tc.tile_pool(name="qkfp", bufs=1) as qkf_pool,
        tc.tile_pool(name="vstp", bufs=1) as v_pool,
    ):
        qkf = [
            qkf_pool.tile([128, S], F32R, tag="qkf", bufs=8, name=f"qkf{t}")
            for t in range(8)
        ]
        v_store = v_pool.tile([128, KT * HPC * VW], F32R, tag="vst", name="vst")
        # fill the per-head ones columns (and everything else) by DMA-
        # broadcasting a [128, VW] ones block over all (seq-tile, head) slots
        ones_bcast = bass.AP(
            tensor=io["ones65"].tensor,
            offset=0,
            ap=[[VW, 128], [0, KT * HPC], [1, VW]],
        )
        nc.sync.dma_start(
            out=v_store.rearrange("p (g u) -> p g u", u=VW), in_=ones_bcast
        )

        # ---- phases 1+2, processed in two sequence halves ---------------
        # qk tile t: even t -> q of head pair t//2, odd t -> k of pair t//2
        # rows 0:64 = first head of the pair, 64:128 = second head.
        with (
            tc.tile_pool(name="xtp", bufs=1) as xt_pool,
            tc.tile_pool(name="ph1c", bufs=1) as cpool,
            tc.tile_pool(name="ph1psA", bufs=1, space="PSUM") as pA_pool,
            tc.tile_pool(name="ph1psB", bufs=1, space="PSUM") as pB_pool,
            tc.tile_pool(name="ph2ps", bufs=1, space="PSUM") as pv_pool,
            tc.tile_pool(name="ph1sb", bufs=1) as ph1_pool,
        ):
            wqk_sb = []
            for c in range(KC):
                w = cpool.tile(
                    [128, 2 * HPC * D], F32R, tag="wqk", bufs=KC, name=f"wqk{c}"
                )
                nc.sync.dma_start(out=w, in_=io["wqkT"][c * 128 : (c + 1) * 128, :])
                wqk_sb.append(w)
            wv_sb = []
            for c in range(KC):
                w = cpool.tile([128, HPC * D], F32R, tag="wv", bufs=KC, name=f"wv{c}")
                nc.sync.dma_start(out=w, in_=io["wvT"][c * 128 : (c + 1) * 128, :])
                wv_sb.append(w)
            r2t_sb = cpool.tile([128, 128], F32R, tag="r2t", name="r2t")
            nc.sync.dma_start(out=r2t_sb, in_=io["r2t"])
            qkb_sb = cpool.tile([128, 8], F32, tag="qkb", name="qkb")
            nc.sync.dma_start(out=qkb_sb, in_=io["qkb"])
            qkbr_sb = cpool.tile([128, 8], F32, tag="qkbr", name="qkbr")
            nc.sync.dma_start(out=qkbr_sb, in_=io["qkbr"])

            for hf in range(2):
                sl = slice(hf * S2, (hf + 1) * S2)
                xt_h = []
                for c in range(KC):
                    t = xt_pool.tile([128, S2], F32R, tag="xt", bufs=KC, name=f"xt{c}")
                    nc.sync.dma_start(out=t, in_=io["xT"][c * 128 : (c + 1) * 128, sl])
                    xt_h.append(t)
                cos2_h = ph1_pool.tile([128, S2], F32, tag="cos2h", bufs=1, name="cos2h")
                nc.sync.dma_start(out=cos2_h, in_=io["cos2"][:, sl])
                sin2_h = ph1_pool.tile([128, S2], F32, tag="sin2h", bufs=1, name="sin2h")
                nc.sync.dma_start(out=sin2_h, in_=io["sin2"][:, sl])

                # qk projection + rope for this half
                for t in range(8):
                    pA = pA_pool.tile([128, S2], F32, tag="pA", bufs=2, name="pA")
                    for c in range(KC):
                        for n0, nn in half_chunks:
                            nc.tensor.matmul(
                                pA[:, n0 : n0 + nn],
                                lhsT=(wqk_sb[c][:, t * 128 : (t + 1) * 128]),
                                rhs=(xt_h[c][:, n0 : n0 + nn]),
                                start=(c == 0),
                                stop=(c == KC - 1),
                                skip_group_check=True,
                            )
                    raw = ph1_pool.tile([128, S2], F32R, tag="raw", bufs=2, name="raw")
                    nc.scalar.copy(raw, pA)
                    pB = pB_pool.tile([128, S2], F32, tag="pB", bufs=1, name="pB")
                    for n0, nn in half_chunks:
                        nc.tensor.matmul(
                            pB[:, n0 : n0 + nn],
                            lhsT=(r2t_sb),
                            rhs=(raw[:, n0 : n0 + nn]),
                            start=True,
                            stop=True,
                        )
                    # qkf = (pA + bias) * cos2 + (pB + rot_bias) * sin2
                    nc.vector.scalar_tensor_tensor(
                        qkf[t][:, sl], pA, qkb_sb[:, t : t + 1], cos2_h,
                        op0=OP.add, op1=OP.mult,
                    )
                    t2 = ph1_pool.tile([128, S2], F32, tag="t2", bufs=1, name="t2")
                    nc.vector.scalar_tensor_tensor(
                        t2, pB, qkbr_sb[:, t : t + 1], sin2_h,
                        op0=OP.add, op1=OP.mult,
                    )
                    nc.vector.tensor_add(qkf[t][:, sl], qkf[t][:, sl], t2.bitcast(F32R))

                # v projection for this half's seq tiles
                for i in range(KT // 2):
                    gi = hf * (KT // 2) + i
                    pv = pv_pool.tile([128, HPC * D], F32, tag="pv", bufs=2, name="pv")
                    for c in range(KC):
                        nc.tensor.matmul(
                            pv,
                            lhsT=(xt_h[c][:, i * 128 : (i + 1) * 128]),
                            rhs=(wv_sb[c]),
                            start=(c == 0),
                            stop=(c == KC - 1),
                            skip_group_check=True,
                        )
                    dst = v_store[:, gi * HPC * VW : (gi + 1) * HPC * VW].rearrange(
                        "p (h u) -> p h u", u=VW
                    )[:, :, 0:D]
                    src = pv.rearrange("p (h u) -> p h u", u=D)
                    nc.vector.tensor_copy(dst, src)

        # ---- phases 3+4 (xt pool released) ------------------------------
        with (
            tc.tile_pool(name="ohp", bufs=1) as oh_pool,
            tc.tile_pool(name="pairp", bufs=1) as pair_pool,
            tc.tile_pool(name="pwp", bufs=1) as pw_pool,
        ):
            out_pair = [
                pair_pool.tile([128, S], F32R, tag="pair", bufs=4, name=f"pair{i}")
                for i in range(4)
            ]
            projw_sb = []
            for kc in range(4):
                t = pw_pool.tile([128, C], F32R, tag="pjw", bufs=4, name=f"pjw{kc}")
                nc.sync.dma_start(out=t, in_=io["projT"][kc * 128 : (kc + 1) * 128, :])
                projw_sb.append(t)

            # ---- phase 3: attention per head ----------------------------
            with (
                tc.tile_pool(name="qkps", bufs=1, space="PSUM") as qkp_pool,
                tc.tile_pool(name="avps", bufs=1, space="PSUM") as av_pool,
                tc.tile_pool(name="attnp", bufs=1) as attn_pool,
                tc.tile_pool(name="divp", bufs=1) as div_pool,
            ):
                for lh in range(HPC):
                    p, hf = lh // 2, lh % 2
                    r0 = hf * 64
                    qT = qkf[2 * p]
                    kT = qkf[2 * p + 1]
                    avp = av_pool.tile([D + 1, S], F32, tag="av", bufs=1, name="av")
                    for i in range(KT):
                        at = attn_pool.tile([128, S], F32R, tag="attn", bufs=2, name="at")
                        for hq in range(2):
                            qkp = qkp_pool.tile(
                                [128, S2], F32, tag="qkp", bufs=2, name="qkp"
                            )
                            for n0, nn in half_chunks:
                                nc.tensor.matmul(
                                    qkp[:, n0 : n0 + nn],
                                    lhsT=(kT[r0 : r0 + 64, i * 128 : (i + 1) * 128]
                                    ),
                                    rhs=(qT[
                                            r0 : r0 + 64,
                                            hq * S2 + n0 : hq * S2 + n0 + nn,
                                        ]
                                    ),
                                    start=True,
                                    stop=True,
                                )
                            nc.scalar.activation(
                                at[:, hq * S2 : (hq + 1) * S2], qkp, AF.Exp,
                                scale=0.125,
                            )
                        vsl = v_store[
                            :, i * HPC * VW + lh * VW : i * HPC * VW + (lh + 1) * VW
                        ]
                        for n0, nn in full_chunks:
                            nc.tensor.matmul(
                                avp[:, n0 : n0 + nn],
                                lhsT=(vsl),
                                rhs=(at[:, n0 : n0 + nn]),
                                start=(i == 0),
                                stop=(i == KT - 1),
                                skip_group_check=True,
                            )
                    # normalize: out = avp[0:64] * (1 / avp[64])
                    outh = oh_pool.tile([64, S], F32R, tag="outh", bufs=2, name="outh")
                    nc.scalar.copy(outh, avp[0:D, :])
                    stmp = div_pool.tile([D + 1, S], F32, tag="stmp", bufs=1, name="stmp")
                    nc.vector.tensor_copy(stmp[D : D + 1, :], avp[D : D + 1, :])
                    stmp0 = div_pool.tile([1, S], F32, tag="stmp0", bufs=1, name="stmp0")
                    nc.sync.dma_start(out=stmp0, in_=stmp[D : D + 1, :])
                    nc.vector.reciprocal_approx_fast(stmp0, stmp0)
                    rbc = div_pool.tile([64, S], F32, tag="rbc", bufs=1, name="rbc")
                    nc.gpsimd.partition_broadcast(rbc, stmp0)
                    nc.vector.tensor_mul(outh, outh, rbc.bitcast(F32R))
                    nc.sync.dma_start(out=out_pair[p][r0 : r0 + 64, :], in_=outh)

            # ---- phase 4: output projection -----------------------------
            with (
                tc.tile_pool(name="yps", bufs=1, space="PSUM") as yp_pool,
                tc.tile_pool(name="ysbp", bufs=1) as ysb_pool,
            ):
                for m in range(8):
                    yp = yp_pool.tile([128, S], F32, tag="yp", bufs=2, name="yp")
                    for kc in range(4):
                        for n0, nn in full_chunks:
                            nc.tensor.matmul(
                                yp[:, n0 : n0 + nn],
                                lhsT=(projw_sb[kc][:, m * 128 : (m + 1) * 128]),
                                rhs=(out_pair[kc][:, n0 : n0 + nn]),
                                start=(kc == 0),
                                stop=(kc == 3),
                                skip_group_check=True,
                            )
                    for hf in range(2):
                        sl = slice(hf * S2, (hf + 1) * S2)
                        ysb = ysb_pool.tile(
                            [128, S2], F32, tag="ysb", bufs=2, name="ysb"
                        )
                        nc.scalar.copy(ysb, yp[:, sl])
                        nc.sync.dma_start(
                            out=io["yT"][m * 128 : (m + 1) * 128, sl], in_=ysb
                        )


def build(S=2048):
    nc = bacc.Bacc("TRN2", target_bir_lowering=False, debug=False)
    io = {
        "xT": nc.dram_tensor("xT", [C, S], F32R, kind="ExternalInput").ap(),
        "wqkT": nc.dram_tensor("wqkT", [C, 2 * HPC * D], F32R, kind="ExternalInput").ap(),
        "wvT": nc.dram_tensor("wvT", [C, HPC * D], F32R, kind="ExternalInput").ap(),
        "projT": nc.dram_tensor("projT", [HPC * D, C], F32R, kind="ExternalInput").ap(),
        "cos2": nc.dram_tensor("cos2", [128, S], F32, kind="ExternalInput").ap(),
        "sin2": nc.dram_tensor("sin2", [128, S], F32, kind="ExternalInput").ap(),
        "r2t": nc.dram_tensor("r2t", [128, 128], F32R, kind="ExternalInput").ap(),
        "ones65": nc.dram_tensor("ones65", [128, VW], F32R, kind="ExternalInput").ap(),
        "qkb": nc.dram_tensor("qkb", [128, 8], F32, kind="ExternalInput").ap(),
        "qkbr": nc.dram_tensor("qkbr", [128, 8], F32, kind="ExternalInput").ap(),
        "yT": nc.dram_tensor("yT", [C, S], F32, kind="ExternalOutput").ap(),
    }
    with tile.TileContext(nc) as tc:
        _emit(tc, io, S)
    nc.compile()
    return nc


def _sigma():
    """rotate-half permutation on 128 rows (two stacked 64-channel heads)."""
    m = np.arange(128)
    return (m // 64) * 64 + (m % 64 + 32) % 64


def make_core_inputs(core, x, qkv_w, q_bias, proj_w, rope_sin, rope_cos):
    """Build the host-side sharded/transposed input dict for one core."""
    S = x.shape[1]
    b, hg = core // 2, core % 2
    f32 = np.float32

    xT = np.ascontiguousarray(x[b].T, dtype=f32)

    blocks = []
    for p in range(4):
        h0 = hg * HPC + 2 * p
        blocks.append(qkv_w[h0 * D : (h0 + 2) * D, :])  # q rows, heads h0, h0+1
        blocks.append(qkv_w[C + h0 * D : C + (h0 + 2) * D, :])  # k rows
    wqkT = np.ascontiguousarray(np.concatenate(blocks, axis=0).T, dtype=f32)

    wvT = np.ascontiguousarray(
        qkv_w[2 * C + hg * HPC * D : 2 * C + (hg + 1) * HPC * D, :].T, dtype=f32
    )
    projT = np.ascontiguousarray(
        proj_w[:, hg * HPC * D : (hg + 1) * HPC * D].T, dtype=f32
    )

    c1 = np.ones((D, S), dtype=f32)
    c1[:, 1:] = rope_cos.T
    cos2 = np.ascontiguousarray(np.vstack([c1, c1]))
    s1 = np.zeros((D, S), dtype=f32)
    s1[:, 1:] = rope_sin.T
    s1[:32, :] *= -1.0
    sin2 = np.ascontiguousarray(np.vstack([s1, s1]))

    sig = _sigma()
    r2t = np.zeros((128, 128), dtype=f32)
    r2t[sig, np.arange(128)] = 1.0

    qkb = np.zeros((128, 8), dtype=f32)
    for p in range(4):
        h0 = hg * HPC + 2 * p
        qkb[:, 2 * p] = q_bias[h0 * D : (h0 + 2) * D]
    qkbr = qkb[sig, :].copy()

    return {
        "xT": xT, "wqkT": wqkT, "wvT": wvT, "projT": projT,
        "cos2": cos2, "sin2": sin2, "r2t": r2t, "qkb": qkb, "qkbr": qkbr,
        "ones65": np.ones((128, VW), dtype=f32),
    }


_PROGRAM = {}


def _get_program(S):
    if S not in _PROGRAM:
        _PROGRAM[S] = build(S)
    return _PROGRAM[S]


def combine_outputs(yT_list, x, v_bias, proj_w, proj_b):
    """Sum per-core partials and add the host-folded bias corrections."""
    S = x.shape[1]
    corr = (
        v_bias.astype(np.float64) @ proj_w.T.astype(np.float64)
        + proj_b.astype(np.float64)
    ).astype(np.float32)
    y = np.empty((B, S, C), dtype=np.float32)
    for b in range(B):
        y[b] = yT_list[2 * b].T + yT_list[2 * b + 1].T + corr
    return y


def kernel(x, qkv_w, q_bias, v_bias, proj_w, proj_b, rope_sin, rope_cos):
    x = np.asarray(x, dtype=np.float32)
    qkv_w = np.asarray(qkv_w, dtype=np.float32)
    q_bias = np.asarray(q_bias, dtype=np.float32)
    v_bias = np.asarray(v_bias, dtype=np.float32)
    proj_w = np.asarray(proj_w, dtype=np.float32)
    proj_b = np.asarray(proj_b, dtype=np.float32)
    rope_sin = np.asarray(rope_sin, dtype=np.float32)
    rope_cos = np.asarray(rope_cos, dtype=np.float32)

    S = x.shape[1]
    in_maps = [
        make_core_inputs(c, x, qkv_w, q_bias, proj_w, rope_sin, rope_cos)
        for c in range(NCORES)
    ]
    nc = _get_program(S)
    res = run_bass_kernel_spmd(nc, in_maps, core_ids=list(range(NCORES)))
    yT_list = [r["yT"] for r in res.results]
    return combine_outputs(yT_list, x, v_bias, proj_w=proj_w, proj_b=proj_b)
